# revision 4
# baseline (speedup 1.0000x reference)
"""Trainium2 Bass kernel for nn_AttentionMM (B=64, T=512, E=512), 8 NeuronCores.

Math (align factored away — O(B*T*E) instead of O(B*T^2*E), numerically
equivalent up to fp32 rounding):
    u1 = tanh(m1 @ W1 + b1)          u2 = tanh(m2 @ W2 + b2)
    g1 = m1^T @ u1                   g2 = m2^T @ u2
    s1 = m1 @ g2                     s2 = m2 @ g1          (= align@u2 / align^T@u1)
    a1 = softmax(s1)                 a2 = softmax(s2)
    v1 = m1^T @ a1                   v2 = m2^T @ a2
    out = concat([v1, v2], -1)

Sharding: pure data parallelism, batch dim 64 -> 8 cores x 8 rows.

Per-core implementation: all matvecs run on the PE in row form
(out[1,512] = vecT @ M) using float32r (full-rate; ~12-bit mantissa, PSUM
accumulation in fp32).  T-contractions stream the natural layout [t,e];
E-contractions stream a PE-transposed copy [e,t].  Intermediate vectors are
staged 4-per-tile at partitions {0,32,64,96} and converted row->column with
batched PE transposes so they can feed the next matmul as stationary columns.
Softmax is fused into ACT: exp(x - max) with accum_out giving the sums; the
1/sum scaling is applied after the v matvec.
"""

import numpy as np

B, T, E = 64, 512, 512
N_CORES = 8
B_LOC = B // N_CORES          # 8 rows per core
GB = 4                        # rows per group (staged 4-up at partitions 32j)
GROUPS = B_LOC // GB

_NC_CACHE = {}


def build_nc(repeat=1):
    import concourse.bacc as bacc
    import concourse.mybir as mybir
    import concourse.tile as tile
    from concourse import masks

    F32 = mybir.dt.float32
    F32R = mybir.dt.float32r
    AF = mybir.ActivationFunctionType
    AX = mybir.AxisListType
    OP = mybir.AluOpType

    nc = bacc.Bacc("TRN2", target_bir_lowering=False, debug=False,
                   num_devices=N_CORES)

    m_d = [nc.dram_tensor("m1", [B_LOC, T, E], F32, kind="ExternalInput"),
           nc.dram_tensor("m2", [B_LOC, T, E], F32, kind="ExternalInput")]
    W_d = [nc.dram_tensor("W1", [B_LOC, E, 1], F32, kind="ExternalInput"),
           nc.dram_tensor("W2", [B_LOC, E, 1], F32, kind="ExternalInput")]
    b_d = [nc.dram_tensor("b1", [B_LOC, T], F32, kind="ExternalInput"),
           nc.dram_tensor("b2", [B_LOC, T], F32, kind="ExternalInput")]
    out_d = nc.dram_tensor("out", [B_LOC, 2 * E], F32, kind="ExternalOutput")

    with tile.TileContext(nc) as tc:
        with (
            tc.tile_pool(name="const", bufs=1) as cpool,
            tc.tile_pool(name="mat", bufs=1) as mpool,     # natural tiles (dbl-buffered)
            tc.tile_pool(name="matT", bufs=1) as tpool,    # transposed tiles
            tc.tile_pool(name="vec", bufs=1) as vpool,     # staging + vector tiles
            tc.tile_pool(name="ps_mt", bufs=2, space="PSUM") as ps_mt,
            tc.tile_pool(name="ps_row", bufs=4, space="PSUM") as ps_row,
            tc.tile_pool(name="ps_conv", bufs=1, space="PSUM") as ps_conv,
        ):
            identF = cpool.tile([128, 128], F32)
            identR = cpool.tile([128, 128], F32R)
            masks.make_identity(nc, identF[:])
            nc.sync.dma_start(identR[:], identF[:].bitcast(F32R))

            for _rep in range(repeat):
                for g in range(GROUPS):
                    # ---------------- load ----------------
                    M = [[None] * GB, [None] * GB]    # natural [128, tc, 512] (part=t)
                    MT = [[None] * GB, [None] * GB]   # transposed (part=e)
                    Wc = [[None] * GB, [None] * GB]   # W columns [128, 4]
                    bias = [None, None]               # [128,512] rows at 32j
                    for s in range(2):
                        bias[s] = vpool.tile([128, T], F32, tag=f"bias{s}", name=f"bias{s}")
                        nc.sync.dma_start(
                            bias[s][:].rearrange("(j r) f -> j r f", r=32)[:, 0, :],
                            b_d[s].ap()[4 * g : 4 * g + 4, :],
                        )
                        for j in range(GB):
                            b = 4 * g + j
                            M[s][j] = mpool.tile([128, 4, E], F32R, tag=f"m{s}{j}", name=f"M{s}{j}", bufs=2 if s == 0 else 1)
                            nc.sync.dma_start(
                                M[s][j][:],
                                m_d[s].ap()[b].rearrange(
                                    "(k p) e -> p k e", p=128).bitcast(F32R),
                            )
                            Wc[s][j] = vpool.tile([128, 4], F32R, tag=f"w{s}{j}", name=f"Wc{s}{j}")
                            nc.sync.dma_start(
                                Wc[s][j][:],
                                W_d[s].ap()[b].rearrange(
                                    "(k p) one -> p (k one)", p=128).bitcast(F32R),
                            )

                    # ---------------- transpose m -> mT ----------------
                    for s in range(2):
                        for j in range(GB):
                            MT[s][j] = tpool.tile([128, 4, T], F32R, tag=f"t{s}{j}", name=f"MT{s}{j}")
                            for ec in range(4):
                                mt_ps = ps_mt.tile([128, T], F32R, tag="mt")
                                for tcH in range(4):
                                    nc.tensor.transpose(
                                        mt_ps[:, tcH * 128:(tcH + 1) * 128],
                                        M[s][j][:, tcH, ec * 128:(ec + 1) * 128],
                                        identR[:],
                                    )
                                nc.any.tensor_copy(MT[s][j][:, ec, :], mt_ps[:])

                    # ---------------- z = m @ W + b ; u = tanh(z) ----------------
                    Ucol = [None, None]
                    for s in range(2):
                        zst = vpool.tile([128, T], F32, tag=f"zst{s}")
                        nc.vector.memset(zst[:], 0.0)
                        for j in range(GB):
                            z_ps = ps_row.tile([1, T], F32, tag="row")
                            for k in range(4):
                                nc.tensor.matmul(
                                    z_ps[:], Wc[s][j][:, k : k + 1],
                                    MT[s][j][:, k, :],
                                    start=(k == 0), stop=(k == 3),
                                )
                            nc.vector.tensor_add(
                                zst[32 * j : 32 * j + 1, :], z_ps[:],
                                bias[s][32 * j : 32 * j + 1, :],
                            )
                        Ucol[s] = vpool.tile([128, 4, 128], F32R, tag=f"u{s}", name=f"Ucol{s}")
                        cps = ps_conv.tile([128, T], F32, tag="conv")
                        for tcH in range(4):
                            nc.tensor.transpose(
                                cps[:, tcH * 128:(tcH + 1) * 128],
                                zst[:, tcH * 128:(tcH + 1) * 128], identF[:],
                            )
                        for tcH in range(4):
                            nc.scalar.activation(
                                Ucol[s][:, tcH, :],
                                cps[:, tcH * 128:(tcH + 1) * 128], AF.Tanh,
                            )

                    # ---------------- g = m^T @ u ----------------
                    Gcol = [None, None]
                    for s in range(2):
                        gst = vpool.tile([128, E], F32, tag=f"gst{s}")
                        nc.vector.memset(gst[:], 0.0)
                        for j in range(GB):
                            g_ps = ps_row.tile([1, E], F32, tag="row")
                            for k in range(4):
                                nc.tensor.matmul(
                                    g_ps[:], Ucol[s][:, k, 32 * j : 32 * j + 1],
                                    M[s][j][:, k, :],
                                    start=(k == 0), stop=(k == 3),
                                )
                            nc.vector.tensor_copy(
                                gst[32 * j : 32 * j + 1, :], g_ps[:])
                        Gcol[s] = vpool.tile([128, 4, 128], F32R, tag=f"g{s}", name=f"Gcol{s}")
                        cps = ps_conv.tile([128, E], F32, tag="conv")
                        for tcH in range(4):
                            nc.tensor.transpose(
                                cps[:, tcH * 128:(tcH + 1) * 128],
                                gst[:, tcH * 128:(tcH + 1) * 128], identF[:],
                            )
                        for tcH in range(4):
                            nc.scalar.activation(
                                Gcol[s][:, tcH, :],
                                cps[:, tcH * 128:(tcH + 1) * 128], AF.Copy,
                            )

                    # ------- s = m @ g_other ; p = exp(s - max), sums -------
                    Acol = [None, None]
                    sums = [[None] * GB, [None] * GB]
                    for s in range(2):
                        pst = vpool.tile([128, T], F32, tag=f"pst{s}")
                        nc.vector.memset(pst[:], 0.0)
                        for j in range(GB):
                            s_ps = ps_row.tile([1, T], F32, tag="row")
                            for k in range(4):
                                nc.tensor.matmul(
                                    s_ps[:], Gcol[1 - s][:, k, 32 * j : 32 * j + 1],
                                    MT[s][j][:, k, :],
                                    start=(k == 0), stop=(k == 3),
                                )
                            negmax = vpool.tile([1, 1], F32, tag="negmax")
                            nc.vector.tensor_reduce(
                                negmax[:], s_ps[:], axis=AX.X, op=OP.max,
                                negate=True)
                            sums[s][j] = vpool.tile([1, 1], F32, tag=f"sum{s}{j}", name=f"sums{s}{j}")
                            nc.scalar.activation(
                                pst[32 * j : 32 * j + 1, :], s_ps[:], AF.Exp,
                                bias=negmax[:], accum_out=sums[s][j][:],
                            )
                        Acol[s] = vpool.tile([128, 4, 128], F32R, tag=f"a{s}", name=f"Acol{s}")
                        cps = ps_conv.tile([128, T], F32, tag="conv")
                        for tcH in range(4):
                            nc.tensor.transpose(
                                cps[:, tcH * 128:(tcH + 1) * 128],
                                pst[:, tcH * 128:(tcH + 1) * 128], identF[:],
                            )
                        for tcH in range(4):
                            nc.scalar.activation(
                                Acol[s][:, tcH, :],
                                cps[:, tcH * 128:(tcH + 1) * 128], AF.Copy,
                            )

                    # ------- v = (m^T @ p) / sum ; store -------
                    for s in range(2):
                        vout = vpool.tile([128, E], F32, tag=f"vout{s}")
                        for j in range(GB):
                            v_ps = ps_row.tile([1, E], F32, tag="row")
                            for k in range(4):
                                nc.tensor.matmul(
                                    v_ps[:], Acol[s][:, k, 32 * j : 32 * j + 1],
                                    M[s][j][:, k, :],
                                    start=(k == 0), stop=(k == 3),
                                )
                            rs = vpool.tile([1, 1], F32, tag="rs")
                            nc.vector.reciprocal(rs[:], sums[s][j][:])
                            nc.vector.tensor_scalar_mul(
                                vout[32 * j : 32 * j + 1, :], v_ps[:], rs[:])
                        nc.sync.dma_start(
                            out_d.ap()[4 * g : 4 * g + 4, s * E : (s + 1) * E],
                            vout[:].rearrange("(j r) f -> j r f", r=32)[:, 0, :],
                        )

    nc.compile()
    return nc


def _get_nc(repeat=1):
    if repeat not in _NC_CACHE:
        _NC_CACHE[repeat] = build_nc(repeat)
    return _NC_CACHE[repeat]


def kernel(m1, m2, W1, b1, W2, b2):
    from concourse.bass_utils import run_bass_kernel_spmd

    nc = _get_nc()
    in_maps = []
    for c in range(N_CORES):
        sl = slice(c * B_LOC, (c + 1) * B_LOC)
        in_maps.append({
            "m1": np.ascontiguousarray(m1[sl]),
            "m2": np.ascontiguousarray(m2[sl]),
            "W1": np.ascontiguousarray(W1[sl]),
            "b1": np.ascontiguousarray(b1[sl]),
            "W2": np.ascontiguousarray(W2[sl]),
            "b2": np.ascontiguousarray(b2[sl]),
        })
    res = run_bass_kernel_spmd(nc, in_maps, core_ids=list(range(N_CORES)))
    return np.concatenate([r["out"] for r in res.results], axis=0)


# revision 5
# speedup vs baseline: 873.9234x; 873.9234x over previous
"""Trainium2 Bass kernel for nn_AttentionMM (B=64, T=512, E=512), 8 NeuronCores.

Math (align factored away — O(B*T*E) instead of O(B*T^2*E), numerically
equivalent up to fp32 rounding):
    u1 = tanh(m1 @ W1 + b1)          u2 = tanh(m2 @ W2 + b2)
    g1 = m1^T @ u1                   g2 = m2^T @ u2
    s1 = m1 @ g2                     s2 = m2 @ g1          (= align@u2 / align^T@u1)
    a1 = softmax(s1)                 a2 = softmax(s2)
    v1 = m1^T @ a1                   v2 = m2^T @ a2
    out = concat([v1, v2], -1)

Sharding: pure data parallelism, batch dim 64 -> 8 cores x 8 rows.

Per-core implementation: all matvecs run on the PE in row form
(out[1,512] = vecT @ M) using float32r (full-rate; ~12-bit mantissa, PSUM
accumulation in fp32).  T-contractions stream the natural layout [t,e];
E-contractions stream a PE-transposed copy [e,t].  Intermediate vectors are
staged 4-per-tile at partitions {0,32,64,96} and converted row->column with
batched PE transposes so they can feed the next matmul as stationary columns.
Softmax is fused into ACT: exp(x - max) with accum_out giving the sums; the
1/sum scaling is applied after the v matvec.
"""

import numpy as np

B, T, E = 64, 512, 512
N_CORES = 8
B_LOC = B // N_CORES          # 8 rows per core
GB = 4                        # rows per group (staged 4-up at partitions 32j)
GROUPS = B_LOC // GB

_NC_CACHE = {}


def build_nc(repeat=1):
    import concourse.bacc as bacc
    import concourse.mybir as mybir
    import concourse.tile as tile
    from concourse import masks

    F32 = mybir.dt.float32
    F32R = mybir.dt.float32r
    AF = mybir.ActivationFunctionType
    AX = mybir.AxisListType
    OP = mybir.AluOpType

    nc = bacc.Bacc("TRN2", target_bir_lowering=False, debug=False,
                   num_devices=N_CORES)

    m_d = [nc.dram_tensor("m1", [B_LOC, T, E], F32, kind="ExternalInput"),
           nc.dram_tensor("m2", [B_LOC, T, E], F32, kind="ExternalInput")]
    W_d = [nc.dram_tensor("W1", [B_LOC, E, 1], F32, kind="ExternalInput"),
           nc.dram_tensor("W2", [B_LOC, E, 1], F32, kind="ExternalInput")]
    b_d = [nc.dram_tensor("b1", [B_LOC, T], F32, kind="ExternalInput"),
           nc.dram_tensor("b2", [B_LOC, T], F32, kind="ExternalInput")]
    out_d = nc.dram_tensor("out", [B_LOC, 2 * E], F32, kind="ExternalOutput")

    with tile.TileContext(nc) as tc:
        with (
            tc.tile_pool(name="const", bufs=1) as cpool,
            tc.tile_pool(name="mat", bufs=1) as mpool,     # natural tiles (dbl-buffered)
            tc.tile_pool(name="matT", bufs=1) as tpool,    # transposed tiles
            tc.tile_pool(name="vec", bufs=1) as vpool,     # staging + vector tiles
            tc.tile_pool(name="ps_mt", bufs=2, space="PSUM") as ps_mt,
            tc.tile_pool(name="ps_row", bufs=4, space="PSUM") as ps_row,
            tc.tile_pool(name="ps_conv", bufs=1, space="PSUM") as ps_conv,
        ):
            identF = cpool.tile([128, 128], F32)
            identR = cpool.tile([128, 128], F32R)
            masks.make_identity(nc, identF[:])
            nc.sync.dma_start(identR[:], identF[:].bitcast(F32R))

            import contextlib
            loop_ctx = (tc.For_i(0, repeat, 1) if repeat > 1
                        else contextlib.nullcontext())
            with loop_ctx:
                for g in range(GROUPS):
                    # ---------------- load ----------------
                    M = [[None] * GB, [None] * GB]    # natural [128, tc, 512] (part=t)
                    MT = [[None] * GB, [None] * GB]   # transposed (part=e)
                    Wc = [[None] * GB, [None] * GB]   # W columns [128, 4]
                    bias = [None, None]               # [128,512] rows at 32j
                    for s in range(2):
                        bias[s] = vpool.tile([128, T], F32, tag=f"bias{s}", name=f"bias{s}")
                        nc.sync.dma_start(
                            bias[s][:].rearrange("(j r) f -> j r f", r=32)[:, 0, :],
                            b_d[s].ap()[4 * g : 4 * g + 4, :],
                        )
                        for j in range(GB):
                            b = 4 * g + j
                            M[s][j] = mpool.tile([128, 4, E], F32R, tag=f"m{s}{j}", name=f"M{s}{j}", bufs=2 if s == 0 else 1)
                            nc.sync.dma_start(
                                M[s][j][:],
                                m_d[s].ap()[b].rearrange(
                                    "(k p) e -> p k e", p=128).bitcast(F32R),
                            )
                            Wc[s][j] = vpool.tile([128, 4], F32R, tag=f"w{s}{j}", name=f"Wc{s}{j}")
                            nc.sync.dma_start(
                                Wc[s][j][:],
                                W_d[s].ap()[b].rearrange(
                                    "(k p) one -> p (k one)", p=128).bitcast(F32R),
                            )

                    # ---------------- transpose m -> mT ----------------
                    for s in range(2):
                        for j in range(GB):
                            MT[s][j] = tpool.tile([128, 4, T], F32R, tag=f"t{s}{j}", name=f"MT{s}{j}")
                            for ec in range(4):
                                mt_ps = ps_mt.tile([128, T], F32R, tag="mt")
                                for tcH in range(4):
                                    nc.tensor.transpose(
                                        mt_ps[:, tcH * 128:(tcH + 1) * 128],
                                        M[s][j][:, tcH, ec * 128:(ec + 1) * 128],
                                        identR[:],
                                    )
                                nc.any.tensor_copy(MT[s][j][:, ec, :], mt_ps[:])

                    # ---------------- z = m @ W + b ; u = tanh(z) ----------------
                    Ucol = [None, None]
                    for s in range(2):
                        zst = vpool.tile([128, T], F32, tag=f"zst{s}")
                        nc.vector.memset(zst[:], 0.0)
                        for j in range(GB):
                            z_ps = ps_row.tile([1, T], F32, tag="row")
                            for k in range(4):
                                nc.tensor.matmul(
                                    z_ps[:], Wc[s][j][:, k : k + 1],
                                    MT[s][j][:, k, :],
                                    start=(k == 0), stop=(k == 3),
                                )
                            nc.vector.tensor_add(
                                zst[32 * j : 32 * j + 1, :], z_ps[:],
                                bias[s][32 * j : 32 * j + 1, :],
                            )
                        Ucol[s] = vpool.tile([128, 4, 128], F32R, tag=f"u{s}", name=f"Ucol{s}")
                        cps = ps_conv.tile([128, T], F32, tag="conv")
                        for tcH in range(4):
                            nc.tensor.transpose(
                                cps[:, tcH * 128:(tcH + 1) * 128],
                                zst[:, tcH * 128:(tcH + 1) * 128], identF[:],
                            )
                        for tcH in range(4):
                            nc.scalar.activation(
                                Ucol[s][:, tcH, :],
                                cps[:, tcH * 128:(tcH + 1) * 128], AF.Tanh,
                            )

                    # ---------------- g = m^T @ u ----------------
                    Gcol = [None, None]
                    for s in range(2):
                        gst = vpool.tile([128, E], F32, tag=f"gst{s}")
                        nc.vector.memset(gst[:], 0.0)
                        for j in range(GB):
                            g_ps = ps_row.tile([1, E], F32, tag="row")
                            for k in range(4):
                                nc.tensor.matmul(
                                    g_ps[:], Ucol[s][:, k, 32 * j : 32 * j + 1],
                                    M[s][j][:, k, :],
                                    start=(k == 0), stop=(k == 3),
                                )
                            nc.vector.tensor_copy(
                                gst[32 * j : 32 * j + 1, :], g_ps[:])
                        Gcol[s] = vpool.tile([128, 4, 128], F32R, tag=f"g{s}", name=f"Gcol{s}")
                        cps = ps_conv.tile([128, E], F32, tag="conv")
                        for tcH in range(4):
                            nc.tensor.transpose(
                                cps[:, tcH * 128:(tcH + 1) * 128],
                                gst[:, tcH * 128:(tcH + 1) * 128], identF[:],
                            )
                        for tcH in range(4):
                            nc.scalar.activation(
                                Gcol[s][:, tcH, :],
                                cps[:, tcH * 128:(tcH + 1) * 128], AF.Copy,
                            )

                    # ------- s = m @ g_other ; p = exp(s - max), sums -------
                    Acol = [None, None]
                    sums = [[None] * GB, [None] * GB]
                    for s in range(2):
                        pst = vpool.tile([128, T], F32, tag=f"pst{s}")
                        nc.vector.memset(pst[:], 0.0)
                        for j in range(GB):
                            s_ps = ps_row.tile([1, T], F32, tag="row")
                            for k in range(4):
                                nc.tensor.matmul(
                                    s_ps[:], Gcol[1 - s][:, k, 32 * j : 32 * j + 1],
                                    MT[s][j][:, k, :],
                                    start=(k == 0), stop=(k == 3),
                                )
                            negmax = vpool.tile([1, 1], F32, tag="negmax")
                            nc.vector.tensor_reduce(
                                negmax[:], s_ps[:], axis=AX.X, op=OP.max,
                                negate=True)
                            sums[s][j] = vpool.tile([1, 1], F32, tag=f"sum{s}{j}", name=f"sums{s}{j}")
                            nc.scalar.activation(
                                pst[32 * j : 32 * j + 1, :], s_ps[:], AF.Exp,
                                bias=negmax[:], accum_out=sums[s][j][:],
                            )
                        Acol[s] = vpool.tile([128, 4, 128], F32R, tag=f"a{s}", name=f"Acol{s}")
                        cps = ps_conv.tile([128, T], F32, tag="conv")
                        for tcH in range(4):
                            nc.tensor.transpose(
                                cps[:, tcH * 128:(tcH + 1) * 128],
                                pst[:, tcH * 128:(tcH + 1) * 128], identF[:],
                            )
                        for tcH in range(4):
                            nc.scalar.activation(
                                Acol[s][:, tcH, :],
                                cps[:, tcH * 128:(tcH + 1) * 128], AF.Copy,
                            )

                    # ------- v = (m^T @ p) / sum ; store -------
                    for s in range(2):
                        vout = vpool.tile([128, E], F32, tag=f"vout{s}")
                        for j in range(GB):
                            v_ps = ps_row.tile([1, E], F32, tag="row")
                            for k in range(4):
                                nc.tensor.matmul(
                                    v_ps[:], Acol[s][:, k, 32 * j : 32 * j + 1],
                                    M[s][j][:, k, :],
                                    start=(k == 0), stop=(k == 3),
                                )
                            rs = vpool.tile([1, 1], F32, tag="rs")
                            nc.vector.reciprocal(rs[:], sums[s][j][:])
                            nc.vector.tensor_scalar_mul(
                                vout[32 * j : 32 * j + 1, :], v_ps[:], rs[:])
                        nc.sync.dma_start(
                            out_d.ap()[4 * g : 4 * g + 4, s * E : (s + 1) * E],
                            vout[:].rearrange("(j r) f -> j r f", r=32)[:, 0, :],
                        )

    nc.compile()
    return nc


def _get_nc(repeat=1):
    if repeat not in _NC_CACHE:
        _NC_CACHE[repeat] = build_nc(repeat)
    return _NC_CACHE[repeat]


def kernel(m1, m2, W1, b1, W2, b2):
    from concourse.bass_utils import run_bass_kernel_spmd

    nc = _get_nc()
    in_maps = []
    for c in range(N_CORES):
        sl = slice(c * B_LOC, (c + 1) * B_LOC)
        in_maps.append({
            "m1": np.ascontiguousarray(m1[sl]),
            "m2": np.ascontiguousarray(m2[sl]),
            "W1": np.ascontiguousarray(W1[sl]),
            "b1": np.ascontiguousarray(b1[sl]),
            "W2": np.ascontiguousarray(W2[sl]),
            "b2": np.ascontiguousarray(b2[sl]),
        })
    res = run_bass_kernel_spmd(nc, in_maps, core_ids=list(range(N_CORES)))
    return np.concatenate([r["out"] for r in res.results], axis=0)
